# revision 3
# baseline (speedup 1.0000x reference)
"""Multi-head attention forward on 8 Trainium2 NeuronCores.

Problem: x[4,2048,1024], W_attn[3072,1024], W_proj[1024,1024], b_proj[1024]
  qkv = x @ W_attn.T ; per-head softmax(q k^T / sqrt(64)) @ v ; out = y @ W_proj.T + b

v2 moves ALL data redistribution on-device to minimize host<->device bytes:
  in:  x 32MB (sharded, no replication) + W 16MB (sharded) + 40KB misc
  out: 32MB (final rows, bias added on device, no host combine)

Sharding: core c <-> (batch b=c//2, half h=c%2). Core c receives x rows
(b, t-half h) and computes attention for head-group hg=c%2 over full T of
its batch, then the pair exchanges normalized y halves so each core does
the full output projection for its own t-half rows only.

On-device dataflow (single NEFF, all static addressing):
  - inputs converted to f16, then collectives: wqk AllGather over
    {evens}/{odds} (host pre-permutes rows so each group gathers exactly
    its head-group's [q;k] rows), x pair AllGather, wv grouped, wp all-8.
  - transposes on the PE (128x128 blocks via identity matmul) produce
    xT/wqkT/wvT/wpT in SBUF.
  - qkv projection -> qkT [o,t] f16 (head pair 2m/2m+1 in rows 0:64/64:128)
    and v in [t, head, d+ones] layout (65th column = softmax denominator).
  - attention per (q-half, head-pair): s^T = k^T q matmuls with two heads
    packed in the PE via tile_position; exp fused with 1/8 scale on ScalarE;
    p@v accumulates y + denominator; y normalized during PSUM unload
    (reciprocal via a DRAM round-trip partition-broadcast) and staged
    straight to DRAM; after each q-half a pair AllGather shares it.
  - epilogue: mask-input-driven select picks this core's t-half from the
    gathered y (no core-id-dependent addressing anywhere), then the output
    projection over all 16 heads' channels + bias, DMA to out.
"""

import sys

import numpy as np

if "/opt/trn_rl_repo" not in sys.path:
    sys.path.insert(0, "/opt/trn_rl_repo")

B, T, C, H, D = 4, 2048, 1024, 16, 64
HPG = H // 2          # heads per core = 8
CL = HPG * D          # per-group y-channels = 512
NCORES = 8

_cache = {}


def _build():
    import concourse.bacc as bacc
    import concourse.bass as bass
    import concourse.masks as masks
    import concourse.mybir as mybir
    import concourse.tile as tile
    from concourse.bass import ds, ts

    f32 = mybir.dt.float32
    f16 = mybir.dt.float16
    EXP = mybir.ActivationFunctionType.Exp

    PAIRS = [[0, 1], [2, 3], [4, 5], [6, 7]]
    HGRPS = [[0, 2, 4, 6], [1, 3, 5, 7]]
    ALL8 = [list(range(8))]

    nc = bacc.Bacc("TRN2", target_bir_lowering=False, debug=False,
                   enable_asserts=False)

    x_sh = nc.dram_tensor("x_sh", [1024, 1024], f32, kind="ExternalInput").ap()
    wqk_sh = nc.dram_tensor("wqk_sh", [256, 1024], f32,
                            kind="ExternalInput").ap()
    wv_sh = nc.dram_tensor("wv_sh", [128, 1024], f32,
                           kind="ExternalInput").ap()
    wp_sh = nc.dram_tensor("wp_sh", [128, 1024], f32,
                           kind="ExternalInput").ap()
    b_sh = nc.dram_tensor("b_sh", [1, 1024], f32, kind="ExternalInput").ap()
    m_sh = nc.dram_tensor("m_sh", [1, 1024], f32, kind="ExternalInput").ap()
    out = nc.dram_tensor("out", [1024, 1024], f32, kind="ExternalOutput").ap()
    rec_dram = nc.dram_tensor("rec_scr", [HPG, T], f32, kind="Internal").ap()

    with tile.TileContext(nc) as tc:
        with tc.tile_pool(name="pers", bufs=1) as pers, \
             tc.tile_pool(name="dram", bufs=1, space="DRAM") as dram:
            # ---------- persistent SBUF ----------
            ones8 = pers.tile([128, HPG], f32, name="ones8")
            nc.vector.memset(ones8, 1.0)
            biasb = pers.tile([128, 1024], f32, name="biasb")
            mskb = pers.tile([128, 1024], mybir.dt.uint8, name="mskb")
            msk32 = pers.tile([128, 1024], f32, name="msk32")
            src = bass.AP(tensor=b_sh.tensor, offset=0,
                          ap=[[0, 128], [1, 1024]])
            nc.gpsimd.dma_start(out=biasb[:], in_=src)
            srcm = bass.AP(tensor=m_sh.tensor, offset=0,
                           ap=[[0, 128], [1, 1024]])
            nc.gpsimd.dma_start(out=msk32[:], in_=srcm)
            nc.vector.tensor_copy(mskb[:], msk32[:])

            xT = [pers.tile([128, T], f16, name=f"xT{k}") for k in range(8)]
            wqkT = [pers.tile([128, 1024], f16, name=f"wqkT{k}")
                    for k in range(8)]
            wvT = [pers.tile([128, CL], f16, name=f"wvT{k}") for k in range(8)]
            wpT = [pers.tile([128, 1024], f16, name=f"wpT{k}")
                   for k in range(8)]
            qkt = [pers.tile([128, T], f16, name=f"qkt{m}") for m in range(8)]
            vbuf = [pers.tile([128, HPG, D + 1], f16, name=f"vb{t}")
                    for t in range(16)]
            ycomb = [pers.tile([128, 1024], f16, name=f"yc{k}")
                     for k in range(8)]

            # ---------- internal DRAM ----------
            xb16 = dram.tile([1024, 1024], f16)
            xg16 = dram.tile([2048, 1024], f16)
            wqkb16 = dram.tile([256, 1024], f16)
            wqkg16 = dram.tile([1024, 1024], f16)
            wvb16 = dram.tile([128, 1024], f16)
            wvg16 = dram.tile([512, 1024], f16)
            wpb16 = dram.tile([128, 1024], f16)
            wpg16 = dram.tile([1024, 1024], f16, addr_space="Shared")
            ya = dram.tile([2, 512, 1024], f16)
            yg = dram.tile([2, 1024, 1024], f16)

            # ---------- phase 0a: f16 conversion + collectives ----------
            with tc.tile_pool(name="cvt", bufs=3) as cvt, \
                 tc.tile_pool(name="cvt16", bufs=3) as cvt16:
                def conv(dst, src_ap, rows):
                    for i in range(rows // 128):
                        st = cvt.tile([128, 1024], f32, name="cst", tag="cst")
                        nc.sync.dma_start(st, src_ap[ts(i, 128), :])
                        st6 = cvt16.tile([128, 1024], f16, name="cst6",
                                         tag="cst6")
                        nc.scalar.copy(st6, st)
                        nc.sync.dma_start(dst[ts(i, 128), :], st6)

                conv(wqkb16, wqk_sh, 256)
                nc.gpsimd.collective_compute(
                    "AllGather", mybir.AluOpType.bypass, replica_groups=HGRPS,
                    ins=[wqkb16.opt()], outs=[wqkg16.opt()])
                conv(xb16, x_sh, 1024)
                nc.gpsimd.collective_compute(
                    "AllGather", mybir.AluOpType.bypass, replica_groups=PAIRS,
                    ins=[xb16.opt()], outs=[xg16.opt()])
                conv(wvb16, wv_sh, 128)
                nc.gpsimd.collective_compute(
                    "AllGather", mybir.AluOpType.bypass, replica_groups=HGRPS,
                    ins=[wvb16.opt()], outs=[wvg16.opt()])
                conv(wpb16, wp_sh, 128)
                nc.gpsimd.collective_compute(
                    "AllGather", mybir.AluOpType.bypass, replica_groups=ALL8,
                    ins=[wpb16.opt()], outs=[wpg16.opt()])

            # ---------- phase 0b: XBAR DMA transposes ----------
            for cc in range(8):
                nc.sync.dma_start_transpose(wqkT[cc][:],
                                            wqkg16[:, ts(cc, 128)])
            for cc in range(8):
                nc.sync.dma_start_transpose(xT[cc][:], xg16[:, ts(cc, 128)])
            for cc in range(8):
                nc.sync.dma_start_transpose(wvT[cc][:], wvg16[:, ts(cc, 128)])
            for cc in range(8):
                nc.sync.dma_start_transpose(wpT[cc][:], wpg16[:, ts(cc, 128)])

            # ---------- phase 1: qkv projection ----------
            with tc.tile_pool(name="p1qk", bufs=2, space="PSUM") as p1qk, \
                 tc.tile_pool(name="p1v", bufs=2, space="PSUM") as p1v:
                for half in range(2):
                    for m in range(8):
                        qps = p1qk.tile([128, 1024], f32, name="qps",
                                        tag="qps")
                        for k in range(8):
                            for nq in range(2):
                                nc.tensor.matmul(
                                    qps[:, ts(nq, 512)],
                                    wqkT[k][:, ts(m, 128)],
                                    xT[k][:, ds(half * 1024 + nq * 512, 512)],
                                    start=(k == 0), stop=(k == 7))
                        nc.scalar.copy(qkt[m][:, ds(half * 1024, 1024)], qps)
                    for tl in range(8):
                        tt = half * 8 + tl
                        vps = p1v.tile([128, 512], f32, name="vps", tag="vps")
                        for k in range(8):
                            nc.tensor.matmul(
                                vps,
                                xT[k][:, ds(half * 1024 + tl * 128, 128)],
                                wvT[k],
                                start=(k == 0), stop=(k == 7))
                        nc.vector.tensor_copy(vbuf[tt][:, :, D:D + 1], ones8)
                        nc.vector.tensor_copy(
                            vbuf[tt][:, :, 0:D],
                            vps.rearrange("p (h d) -> p h d", d=D))

            # ---------- phase 2: attention (q-half outer for overlap) ----
            with tc.tile_pool(name="p2s", bufs=4, space="PSUM") as p2s, \
                 tc.tile_pool(name="p2y", bufs=4, space="PSUM") as p2y, \
                 tc.tile_pool(name="p2e", bufs=6) as p2e, \
                 tc.tile_pool(name="p2den", bufs=2) as p2den, \
                 tc.tile_pool(name="p2bc", bufs=4) as p2bc, \
                 tc.tile_pool(name="p2st", bufs=4) as p2st:
                for qc in range(2):
                    for j in range(4):      # head pair (2j, 2j+1)
                        denb = p2den.tile([2, 1024], f32, name="denb",
                                          tag="denb")
                        yps = [[p2y.tile([65, 512], f32, name=f"yps{hh}_{n}",
                                         tag="yps") for n in range(2)]
                               for hh in range(2)]
                        for tt in range(16):
                            for n in range(2):
                                qsl = ds(qc * 1024 + n * 512, 512)
                                spsA = p2s.tile([128, 512], f32, name="spsA",
                                                tag="sps")
                                spsB = p2s.tile([128, 512], f32, name="spsB",
                                                tag="sps")
                                nc.tensor.matmul(
                                    spsA, qkt[4 + j][0:64, ts(tt, 128)],
                                    qkt[j][0:64, qsl],
                                    start=True, stop=True,
                                    tile_position=(0, 0))
                                nc.tensor.matmul(
                                    spsB, qkt[4 + j][64:128, ts(tt, 128)],
                                    qkt[j][64:128, qsl],
                                    start=True, stop=True,
                                    tile_position=(64, 0))
                                expA = p2e.tile([128, 512], f16, name="expA",
                                                tag="exp")
                                expB = p2e.tile([128, 512], f16, name="expB",
                                                tag="exp")
                                nc.scalar.activation(expA, spsA, EXP,
                                                     scale=0.125)
                                nc.scalar.activation(expB, spsB, EXP,
                                                     scale=0.125)
                                nc.tensor.matmul(
                                    yps[0][n][0:65, :],
                                    vbuf[tt][:, 2 * j, 0:D + 1], expA,
                                    start=(tt == 0), stop=(tt == 15))
                                nc.tensor.matmul(
                                    yps[1][n][0:65, :],
                                    vbuf[tt][:, 2 * j + 1, 0:D + 1], expB,
                                    start=(tt == 0), stop=(tt == 15))
                        # denominators -> reciprocal -> DRAM
                        for hh in range(2):
                            for n in range(2):
                                stg = p2st.tile([128, 512], f32, name="stg",
                                                tag="stg")
                                nc.vector.tensor_copy(
                                    stg[64:65, :], yps[hh][n][64:65, :])
                                nc.sync.dma_start(
                                    denb[hh:hh + 1, ts(n, 512)],
                                    stg[64:65, :])
                        recsb = p2den.tile([2, 1024], f32, name="recsb",
                                           tag="recsb")
                        nc.vector.reciprocal_approx_fast(recsb, denb)
                        nc.sync.dma_start(
                            rec_dram[2 * j:2 * j + 2, ds(qc * 1024, 1024)],
                            recsb)
                        # normalized unload straight to ya[qc] DRAM
                        for hh in range(2):
                            for n in range(2):
                                bc = p2bc.tile([128, 512], f32, name="bc",
                                               tag="bc")
                                rsrc = bass.AP(
                                    tensor=rec_dram.tensor,
                                    offset=(2 * j + hh) * T + qc * 1024
                                    + n * 512,
                                    ap=[[0, 64], [1, 512]])
                                nc.gpsimd.dma_start(out=bc[0:64, :], in_=rsrc)
                                sty = p2st.tile([128, 512], f16, name="sty",
                                                tag="sty")
                                nc.vector.tensor_mul(
                                    sty[0:64, :], yps[hh][n][0:64, :],
                                    bc[0:64, :])
                                nc.sync.dma_start(
                                    ya[qc][ds(128 * j + 64 * hh, 64),
                                           ts(n, 512)],
                                    sty[0:64, :])
                    nc.gpsimd.collective_compute(
                        "AllGather", mybir.AluOpType.bypass,
                        replica_groups=PAIRS,
                        ins=[ya[qc].opt()], outs=[yg[qc].opt()])

            # ---------- phase 3: select own half + output projection ----
            with tc.tile_pool(name="p3a", bufs=4) as p3a, \
                 tc.tile_pool(name="p3o", bufs=3) as p3o, \
                 tc.tile_pool(name="p3ps", bufs=3, space="PSUM") as p3ps:
                for k in range(8):
                    a0 = p3a.tile([128, 1024], f16, name="a0", tag="ya")
                    a1 = p3a.tile([128, 1024], f16, name="a1", tag="ya")
                    nc.sync.dma_start(a0, yg[0][ts(k, 128), :])
                    nc.sync.dma_start(a1, yg[1][ts(k, 128), :])
                    nc.vector.select(ycomb[k][:], mskb[:], a1, a0)
                for tm in range(8):
                    ops = p3ps.tile([128, 1024], f32, name="ops", tag="ops")
                    for k in range(8):
                        for n in range(2):
                            nc.tensor.matmul(
                                ops[:, ts(n, 512)],
                                ycomb[k][:, ts(tm, 128)],
                                wpT[k][:, ts(n, 512)],
                                start=(k == 0), stop=(k == 7))
                    osb = p3o.tile([128, 1024], f32, name="osb", tag="osb")
                    nc.vector.tensor_add(osb, ops, biasb)
                    nc.sync.dma_start(out[ts(tm, 128), :], osb)

    nc.compile()
    return nc


def _get_nc():
    if "nc" not in _cache:
        _cache["nc"] = _build()
    return _cache["nc"]


def _host_prep(x, W_attn, W_proj, b_proj):
    """Global arrays whose natural 8-way row-sharding feeds each core."""
    x = np.asarray(x, dtype=np.float32)
    W_attn = np.asarray(W_attn, dtype=np.float32)
    W_proj = np.asarray(W_proj, dtype=np.float32)
    b_proj = np.asarray(b_proj, dtype=np.float32)
    x2 = x.reshape(NCORES * 1024, 1024)
    # row-permute so grouped AllGathers {evens}/{odds} reconstruct each
    # head-group's [q;k] / [v] rows contiguously
    wqk = np.ascontiguousarray(
        W_attn[:2 * C].reshape(2, 2, 2, 256, 1024)
        .transpose(0, 2, 1, 3, 4).reshape(2 * C, 1024))
    wv = np.ascontiguousarray(
        W_attn[2 * C:].reshape(2, 4, 128, 1024)
        .transpose(1, 0, 2, 3).reshape(C, 1024))
    bias = np.tile(b_proj.reshape(1, 1024), (NCORES, 1))
    if "mask" not in _cache:
        _cache["mask"] = np.repeat(
            np.arange(NCORES, dtype=np.float32)[:, None] % 2, 1024, axis=1)
    return [x2, wqk, wv, W_proj, bias, _cache["mask"]]


def make_in_maps(x, W_attn, W_proj, b_proj):
    g = _host_prep(x, W_attn, W_proj, b_proj)
    names = ["x_sh", "wqk_sh", "wv_sh", "wp_sh", "b_sh", "m_sh"]
    maps = []
    for c in range(NCORES):
        m = {}
        for nm, arr in zip(names, g):
            rows = arr.shape[0] // NCORES
            m[nm] = np.ascontiguousarray(arr[c * rows:(c + 1) * rows])
        maps.append(m)
    return maps


def combine(results):
    return np.concatenate([r["out"] for r in results],
                          axis=0).reshape(B, T, C)


def _get_fn():
    """Jitted SPMD executor: one bass_exec custom call over the 8-core mesh,
    with output buffers cached on device (the kernel writes every element)."""
    if "fn" in _cache:
        return _cache["fn"]
    import jax
    import jax.numpy as jnp
    from jax.sharding import Mesh, NamedSharding, PartitionSpec

    from concourse import bass2jax as b2j
    import concourse.mybir as mybir

    try:
        from jax.experimental.shard_map import shard_map
    except ImportError:
        from jax.shard_map import shard_map

    b2j.install_neuronx_cc_hook()
    nc = _get_nc()
    part_name = nc.partition_id_tensor.name if nc.partition_id_tensor else None
    in_names, out_names, out_avals = [], [], []
    for alloc in nc.m.functions[0].allocations:
        if not isinstance(alloc, mybir.MemoryLocationSet):
            continue
        name = alloc.memorylocations[0].name
        if alloc.kind == "ExternalInput":
            if name != part_name:
                in_names.append(name)
        elif alloc.kind == "ExternalOutput":
            out_names.append(name)
            out_avals.append(jax.core.ShapedArray(tuple(alloc.tensor_shape),
                                                  mybir.dt.np(alloc.dtype)))
    assert in_names == ["x_sh", "wqk_sh", "wv_sh", "wp_sh", "b_sh", "m_sh"], \
        in_names
    assert out_names == ["out"]
    all_in = list(in_names) + list(out_names)
    if part_name is not None:
        all_in.append(part_name)

    def _body(*args):
        operands = list(args)
        if part_name is not None:
            operands.append(b2j.partition_id_tensor())
        return tuple(b2j._bass_exec_p.bind(
            *operands, out_avals=tuple(out_avals), in_names=tuple(all_in),
            out_names=tuple(out_names), lowering_input_output_aliases=(),
            sim_require_finite=True, sim_require_nnan=True, nc=nc))

    devices = jax.devices()[:NCORES]
    mesh = Mesh(np.asarray(devices), ("core",))
    sharding = NamedSharding(mesh, PartitionSpec("core"))
    fn = jax.jit(
        shard_map(_body, mesh=mesh,
                  in_specs=(PartitionSpec("core"),) * 7,
                  out_specs=(PartitionSpec("core"),),
                  check_rep=False),
        keep_unused=True)
    zeros = jax.jit(lambda: jnp.zeros((NCORES * 1024, 1024), jnp.float32),
                    out_shardings=sharding)()
    zeros.block_until_ready()
    state = {"fn": fn, "sharding": sharding, "zeros": zeros}
    _cache["fn"] = state
    return state


def kernel(x, W_attn, W_proj, b_proj):
    import jax

    g = _host_prep(x, W_attn, W_proj, b_proj)
    try:
        st = _get_fn()
        if "mask_dev" not in _cache:
            _cache["mask_dev"] = jax.device_put(g[5], st["sharding"])
        dev = jax.device_put(g[:5], [st["sharding"]] * 5)
        (res,) = st["fn"](*dev, _cache["mask_dev"], st["zeros"])
        return np.asarray(res).reshape(B, T, C)
    except Exception:
        pass
    from concourse import bass2jax as b2j
    b2j.install_neuronx_cc_hook()
    in_maps = make_in_maps(x, W_attn, W_proj, b_proj)
    results = b2j.run_bass_via_pjrt(_get_nc(), in_maps, n_cores=NCORES)
    return combine(results)
